# revision 28
# baseline (speedup 1.0000x reference)
"""Trainium2 Bass kernel for MixformerAttention (sparse attention), v3.

Problem shape (hardcoded from the problem spec):
  x [B=64, N=320, C=768], W_qkv [768, 2304], W_proj [768, 768], b_proj [768]
  H=12 heads, Dh=64, template length L = t_h*t_w = 64, search = 256.

Sharding: data-parallel over batch across 8 NeuronCores (8 batches/core).

Per-core pipeline (batches processed in pairs of 2 -> 640 tokens = 5x128):
  1. DMA x pair -> SBUF, PE-transpose to x^T (C on partitions), cast fp16.
     The next pair's load+transposes are emitted mid-pair (software pipeline)
     so the PE never sits in a long transpose-only or DMA-wait stretch.
  2. q^T,k^T matmuls (features on partitions); per chunk fc the heads
     (2fc, 2fc+1) sit stacked on partitions 0:64 / 64:128.
     V natural matmul -> v [tok, head, 1+64] with a LEADING ones column.
  3. Scores S^T = k q^T per head, ROW-PAIRED: even head on PE row-group 0,
     odd head on row-group 64 (explicit tile_position) -> 2x concurrency.
     exp via ACT (scale=1/8) -> es fp16 (head order permuted per quad).
  4. PV transposed: O^T[d,q] = [1|v]^T @ expS^T per head -> psum [65, 320]
     (template cols 0:64, search 64:320); row 0 = softmax denominator.
     ACT-copy po -> SBUF (releases the psum bank fast), gpsimd-broadcast
     the denominator row, DVE reciprocal_approx_fast, DVE multiply, then
     DMA partition-shift rows 1:65 into attn^T rows 0:64 / 64:128.
  5. attn^T is directly the proj lhsT: out = attn^T.T @ W_proj + bias,
     5x128-token chunks per pair -> DMA out fp32.

All matmuls fp16 operands (1 cycle/row on PE, fp32 PSUM accumulate).
"""

import contextlib
import functools

import numpy as np

import concourse.bacc as bacc
import concourse.mybir as mybir
from concourse.bass_utils import run_bass_kernel_spmd
from concourse.masks import make_identity
from concourse.tile import TileContext

F32 = mybir.dt.float32
F16 = mybir.dt.float16

NCORES = 8
B, N, C = 64, 320, 768
H, DH = 12, 64
KS = C // 128  # 6 contraction subtiles
B_CORE = B // NCORES  # 8 batches per core
PAIR_TOK = 2 * N  # 640
NPAIR = B_CORE // 2  # 4
TOK_CORE = B_CORE * N  # 2560
L = 64  # template length
S = N - L  # search length 256

# key chunks of one batch's 320 tokens
KT_CHUNKS = [(0, 128), (128, 128), (256, 64)]

# es physical position within a quad of heads {4g..4g+3}: head 4g+r -> pos
QPOS = [0, 2, 1, 3]


def pos_of(h):
    return 4 * (h // 4) + QPOS[h % 4]


def tpos_of(h):
    """Template-es position: evens at 0..5, odds at 6..11."""
    return (h // 2) if h % 2 == 0 else 6 + h // 2


def _load_x(nc, pools, x_ap, p):
    x_nat = pools["x_nat"].tile([128, 5, C], F32, tag="x_nat")
    for t in range(5):
        r0 = p * PAIR_TOK + t * 128
        nc.sync.dma_start(x_nat[:, t, :], x_ap[r0 : r0 + 128, :])
    return x_nat


def _emit_xT(nc, pools, ident32, x_nat):
    """PE-transpose x pair -> x^T fp16 [128, KS, 640]."""
    big2 = pools["big2"]
    xT = pools["xT"].tile([128, KS, PAIR_TOK], F16, tag="xT")
    for fc in range(KS):
        ps = big2.tile([128, PAIR_TOK], F32, tag="big2")
        for t in range(5):
            nc.tensor.transpose(
                ps[:, t * 128 : (t + 1) * 128],
                x_nat[:, t, fc * 128 : (fc + 1) * 128],
                ident32,
            )
        if fc % 2 == 0:
            nc.scalar.copy(xT[:, fc, :], ps[:, :])
        else:
            nc.vector.tensor_copy(xT[:, fc, :], ps[:, :])
    return xT


def _emit_qk(nc, pools, wqkv16, xT):
    """q^T / k^T: features on partitions, heads (2fc, 2fc+1) stacked."""
    big2 = pools["big2"]
    qkT = pools["qkT"].tile([128, 2 * KS, PAIR_TOK], F16, tag="qkT")
    for fc in range(2 * KS):
        ps = big2.tile([128, 1024], F32, tag="big2")
        for ks in range(KS):
            lhsT = wqkv16[:, ks, fc * 128 : (fc + 1) * 128]
            nc.tensor.matmul(
                ps[:, 0:320],
                lhsT=lhsT,
                rhs=xT[:, ks, 0:320],
                start=(ks == 0),
                stop=(ks == KS - 1),
            )
            nc.tensor.matmul(
                ps[:, 512:832],
                lhsT=lhsT,
                rhs=xT[:, ks, 320:640],
                start=(ks == 0),
                stop=(ks == KS - 1),
            )
        nc.scalar.copy(
            qkT[:, fc, :].rearrange("p (b x) -> p b x", b=2),
            ps[:, :].rearrange("p (b x) -> p b x", b=2)[:, :, 0:320],
        )
    return qkT


def _emit_batch(nc, pools, wqkv16, xT, qkT, attnT, btok):
    """Scores + exp + V + transposed PV + normalize for one batch."""
    big2 = pools["big2"]
    pop = pools["po"]

    # ---- v natural [tok, h, 0:65]: ones column FIRST, v at 1:65 ----
    va = pools["va"].tile([128, 3, H, 66], F16, tag="va")
    for ci, (koff, ksz) in enumerate(KT_CHUNKS):
        ps = big2.tile([128, C], F32, tag="big2")
        for half, (n0, nw) in enumerate([(0, 512), (512, 256)]):
            for ks in range(KS):
                nc.tensor.matmul(
                    ps[:ksz, n0 : n0 + nw],
                    lhsT=xT[:, ks, btok + koff : btok + koff + ksz],
                    rhs=wqkv16[:, ks, 2 * C + n0 : 2 * C + n0 + nw],
                    start=(ks == 0),
                    stop=(ks == KS - 1),
                )
        if ci % 2 == 0:
            nc.vector.tensor_copy(
                va[:ksz, ci, :, 1:65],
                ps[:ksz, 0:768].rearrange("p (h d) -> p h d", d=64),
            )
        else:
            nc.scalar.copy(
                va[:ksz, ci, :, 1:65],
                ps[:ksz, 0:768].rearrange("p (h d) -> p h d", d=64),
            )
    nc.vector.memset(va[:, :, :, 0], 1.0)

    # ---- template scores first (esm is one cheap ACT op, needed by PV) ----
    # esm positions: evens 0..5, odds 6..11
    esm = pools["esm"].tile([64, H, L], F16, tag="esm")
    psm = big2.tile([128, 1024], F32, tag="big2")
    for j in range(KS):
        for par in range(2):
            h = 2 * j + par
            r0 = 64 * par
            dst0 = 512 * par + 64 * j
            nc.tensor.matmul(
                psm[0:64, dst0 : dst0 + 64],
                lhsT=qkT[r0 : r0 + 64, KS + j, btok : btok + L],
                rhs=qkT[r0 : r0 + 64, j, btok : btok + L],
                start=True,
                stop=True,
                tile_position=(r0, 0),
            )
    nc.scalar.activation(
        esm[:, :, :].rearrange("p (b h) q -> p b h q", b=2),
        psm[0:64, :]
        .rearrange("p (b x) -> p b x", b=2)[:, :, 0:384]
        .rearrange("p b (h q) -> p b h q", q=L),
        mybir.ActivationFunctionType.Exp,
        scale=0.125,
    )

    # ---- per head quad: search scores (row-paired) + exp, then PV ----
    # es layout [keys, ci, pos, 256] with pos = quad-permuted head order
    es = pools["es"].tile([128, 3, H, S], F16, tag="es")
    for g in range(3):
        for ci, (koff, ksz) in enumerate(KT_CHUNKS):
            ps = big2.tile([128, 1024], F32, tag="big2")
            for j in range(2):  # dual-issue slots within the quad
                for par in range(2):  # row-group parity
                    h = 4 * g + 2 * j + par
                    fc = h // 2
                    r0 = 64 * par
                    dst0 = QPOS[h % 4] * 256
                    nc.tensor.matmul(
                        ps[:ksz, dst0 : dst0 + 256],
                        lhsT=qkT[
                            r0 : r0 + 64, KS + fc, btok + koff : btok + koff + ksz
                        ],
                        rhs=qkT[r0 : r0 + 64, fc, btok + L : btok + N],
                        start=True,
                        stop=True,
                        tile_position=(r0, 0),
                    )
            nc.scalar.activation(
                es[:ksz, ci, 4 * g : 4 * g + 4, :],
                ps[:ksz, :].rearrange("p (h q) -> p h q", q=S),
                mybir.ActivationFunctionType.Exp,
                scale=0.125,
            )

        # ---- PV transposed + normalize for the 4 heads of this quad ----
        for h in range(4 * g, 4 * g + 4):
            fc = h // 2
            par = h % 2
            po = pop.tile([65, N], F32, tag="po")
            # template part: cols 0:64 (opens the accumulation group)
            nc.tensor.matmul(
                po[:, 0:L],
                lhsT=va[0:64, 0, h, 0:65],
                rhs=esm[:, tpos_of(h), :],
                start=True,
                stop=False,
            )
            # search part: cols 64:320, accumulated over key chunks
            for ci, (koff, ksz) in enumerate(KT_CHUNKS):
                nc.tensor.matmul(
                    po[:, L:N],
                    lhsT=va[0:ksz, ci, h, 0:65],
                    rhs=es[0:ksz, ci, pos_of(h), :],
                    start=False,
                    stop=(ci == 2),
                )
            # evacuate po quickly so the psum bank frees for the next head;
            # alternate engines so neither queue's latency gates the release
            poc = pools["poc"].tile([65, N], F32, tag="poc")
            if par == 0:
                nc.scalar.copy(poc[:, :], po[:, :])
            else:
                nc.vector.tensor_copy(poc[:, :], po[:, :])
            # denominator row 0: gpsimd-broadcast, fast reciprocal, multiply
            bc = pools["bc"].tile([65, N], F32, tag="bc")
            nc.gpsimd.partition_broadcast(bc[:, :], poc[0:1, :])
            rcpb = pools["rcpb"].tile([65, N], F32, tag="rcpb")
            nc.vector.reciprocal_approx_fast(rcpb[:, :], bc[:, :])
            tmp = pools["tmp"].tile([65, N], F16, tag="tmp")
            nc.vector.tensor_tensor(
                tmp[0:65, :], poc[0:65, :], rcpb[0:65, :], mybir.AluOpType.mult
            )
            nc.sync.dma_start(
                attnT[64 * par : 64 * par + 64, fc, btok : btok + N], tmp[1:65, :]
            )


def _emit_proj(nc, pools, wproj16, bias_bc, attnT, out_ap, p, chunks):
    """Output projection + bias for one pair (chunks of 128 tokens)."""
    big2 = pools["big2"]
    for tch in chunks:
        pp = big2.tile([128, C], F32, tag="big2")
        for half, (n0, nw) in enumerate([(0, 512), (512, 256)]):
            for ks in range(KS):
                nc.tensor.matmul(
                    pp[:, n0 : n0 + nw],
                    lhsT=attnT[:, ks, tch * 128 : (tch + 1) * 128],
                    rhs=wproj16[:, ks, n0 : n0 + nw],
                    start=(ks == 0),
                    stop=(ks == KS - 1),
                )
        out_sb = pools["out"].tile([128, C], F32, tag="out")
        nc.vector.tensor_tensor(
            out_sb[:, :], pp[:, :], bias_bc[:, :], mybir.AluOpType.add
        )
        row0 = p * PAIR_TOK + tch * 128
        nc.sync.dma_start(out_ap[row0 : row0 + 128, :], out_sb[:, :])


def build_kernel():
    nc = bacc.Bacc("TRN2", target_bir_lowering=False)
    x_t = nc.dram_tensor("x", [TOK_CORE, C], F32, kind="ExternalInput")
    wqkv_t = nc.dram_tensor("W_qkv", [C, 3 * C], F32, kind="ExternalInput")
    wproj_t = nc.dram_tensor("W_proj", [C, C], F32, kind="ExternalInput")
    bias_t = nc.dram_tensor("b_proj", [C], F32, kind="ExternalInput")
    out_t = nc.dram_tensor("out", [TOK_CORE, C], F32, kind="ExternalOutput")

    with TileContext(nc) as tc:
        with contextlib.ExitStack() as ctx:
            pools = {
                "const": ctx.enter_context(tc.tile_pool(name="const", bufs=1)),
                "stage": ctx.enter_context(tc.tile_pool(name="stage", bufs=2)),
                "x_nat": ctx.enter_context(tc.tile_pool(name="x_nat", bufs=1)),
                "xT": ctx.enter_context(tc.tile_pool(name="xT", bufs=2)),
                "qkT": ctx.enter_context(tc.tile_pool(name="qkT", bufs=2)),
                "va": ctx.enter_context(tc.tile_pool(name="va", bufs=2)),
                "es": ctx.enter_context(tc.tile_pool(name="es", bufs=2)),
                "esm": ctx.enter_context(tc.tile_pool(name="esm", bufs=2)),
                "attnT": ctx.enter_context(tc.tile_pool(name="attnT", bufs=2)),
                "out": ctx.enter_context(tc.tile_pool(name="out", bufs=3)),
                "poc": ctx.enter_context(tc.tile_pool(name="poc", bufs=5)),
                "bc": ctx.enter_context(tc.tile_pool(name="bc", bufs=5)),
                "tmp": ctx.enter_context(tc.tile_pool(name="tmp", bufs=6)),
                "rcpb": ctx.enter_context(tc.tile_pool(name="rcpb", bufs=4)),
                "big2": ctx.enter_context(
                    tc.tile_pool(name="big2", bufs=3, space="PSUM")
                ),
                "po": ctx.enter_context(
                    tc.tile_pool(name="po", bufs=2, space="PSUM")
                ),
            }
            const = pools["const"]

            wqkv16 = const.tile([128, KS, 3 * C], F16, tag="wqkv16")
            wproj16 = const.tile([128, KS, C], F16, tag="wproj16")
            bias_bc = const.tile([128, C], F32, tag="bias_bc")
            ident32 = const.tile([128, 128], F32, tag="ident32")
            make_identity(nc, ident32)
            x0 = _load_x(nc, pools, x_t.ap(), 0)

            # stage W_qkv by 128-column blocks so the first qk matmul only
            # waits for one ~1.2MB block instead of the whole 7MB weight
            for fb in range(18):
                st = pools["stage"].tile([128, KS, 128], F32, tag="stage")
                nc.sync.dma_start(
                    st[:],
                    wqkv_t.ap()[:, fb * 128 : (fb + 1) * 128].rearrange(
                        "(k p) f -> p k f", p=128
                    ),
                )
                if fb % 2 == 0:
                    nc.scalar.copy(
                        wqkv16[:, :, fb * 128 : (fb + 1) * 128], st[:]
                    )
                else:
                    nc.vector.tensor_copy(
                        wqkv16[:, :, fb * 128 : (fb + 1) * 128], st[:]
                    )
            # software pipeline: transposes of pair p+1 are emitted between
            # batch 0 and batch 1 of pair p; wproj/bias staging is emitted
            # after the first batch so it does not crowd the startup
            xT_next = _emit_xT(nc, pools, ident32, x0)
            for p in range(NPAIR):
                xT = xT_next
                qkT = _emit_qk(nc, pools, wqkv16, xT)
                attnT = pools["attnT"].tile([128, KS, PAIR_TOK], F16, tag="attnT")
                _emit_batch(nc, pools, wqkv16, xT, qkT, attnT, 0)
                if p == 0:
                    for ks in range(KS):
                        stp = pools["stage"].tile([128, C], F32, tag="stagep")
                        nc.sync.dma_start(
                            stp[:], wproj_t.ap()[ks * 128 : (ks + 1) * 128, :]
                        )
                        if ks % 2 == 0:
                            nc.scalar.copy(wproj16[:, ks, :], stp[:])
                        else:
                            nc.vector.tensor_copy(wproj16[:, ks, :], stp[:])
                    brow = pools["stage"].tile([1, C], F32, tag="stagep")
                    nc.sync.dma_start(brow[0:1, :], bias_t.ap().unsqueeze(0))
                    nc.gpsimd.partition_broadcast(bias_bc[:, :], brow[0:1, :])
                if p + 1 < NPAIR:
                    xn = _load_x(nc, pools, x_t.ap(), p + 1)
                    xT_next = _emit_xT(nc, pools, ident32, xn)
                _emit_batch(nc, pools, wqkv16, xT, qkT, attnT, N)
                _emit_proj(
                    nc, pools, wproj16, bias_bc, attnT, out_t.ap(), p,
                    (0, 1, 2, 3, 4),
                )

    nc.compile()
    return nc


@functools.cache
def _get_nc():
    return build_kernel()


def kernel(**inputs):
    x = np.ascontiguousarray(np.asarray(inputs["x"], dtype=np.float32))
    wqkv = np.ascontiguousarray(np.asarray(inputs["W_qkv"], dtype=np.float32))
    wproj = np.ascontiguousarray(np.asarray(inputs["W_proj"], dtype=np.float32))
    bias = np.ascontiguousarray(np.asarray(inputs["b_proj"], dtype=np.float32))
    t_h = int(inputs.get("t_h", 8))
    t_w = int(inputs.get("t_w", 8))
    assert t_h * t_w == 64, "kernel built for template length 64"
    assert x.shape == (B, N, C)

    nc = _get_nc()
    in_maps = [
        {
            "x": x[c * B_CORE : (c + 1) * B_CORE].reshape(TOK_CORE, C),
            "W_qkv": wqkv,
            "W_proj": wproj,
            "b_proj": bias,
        }
        for c in range(NCORES)
    ]
    res = run_bass_kernel_spmd(nc, in_maps, core_ids=list(range(NCORES)))
    out = np.concatenate(
        [r["out"].reshape(B_CORE, N, C) for r in res.results], axis=0
    )
    return out.astype(np.float32)


if __name__ == "__main__":
    _get_nc()
    print("kernel built OK")


# revision 29
# speedup vs baseline: 1.0171x; 1.0171x over previous
"""Trainium2 Bass kernel for MixformerAttention (sparse attention), v3.

Problem shape (hardcoded from the problem spec):
  x [B=64, N=320, C=768], W_qkv [768, 2304], W_proj [768, 768], b_proj [768]
  H=12 heads, Dh=64, template length L = t_h*t_w = 64, search = 256.

Sharding: data-parallel over batch across 8 NeuronCores (8 batches/core).

Per-core pipeline (batches processed in pairs of 2 -> 640 tokens = 5x128):
  1. DMA x pair -> SBUF, PE-transpose to x^T (C on partitions), cast fp16.
     The next pair's load+transposes are emitted mid-pair (software pipeline)
     so the PE never sits in a long transpose-only or DMA-wait stretch.
  2. q^T,k^T matmuls (features on partitions); per chunk fc the heads
     (2fc, 2fc+1) sit stacked on partitions 0:64 / 64:128.
     V natural matmul -> v [tok, head, 1+64] with a LEADING ones column.
  3. Scores S^T = k q^T per head, ROW-PAIRED: even head on PE row-group 0,
     odd head on row-group 64 (explicit tile_position) -> 2x concurrency.
     exp via ACT (scale=1/8) -> es fp16 (head order permuted per quad).
  4. PV transposed: O^T[d,q] = [1|v]^T @ expS^T per head -> psum [65, 320]
     (template cols 0:64, search 64:320); row 0 = softmax denominator.
     ACT-copy po -> SBUF (releases the psum bank fast), gpsimd-broadcast
     the denominator row, DVE reciprocal_approx_fast, DVE multiply, then
     DMA partition-shift rows 1:65 into attn^T rows 0:64 / 64:128.
  5. attn^T is directly the proj lhsT: out = attn^T.T @ W_proj + bias,
     5x128-token chunks per pair -> DMA out fp32.

All matmuls fp16 operands (1 cycle/row on PE, fp32 PSUM accumulate).
"""

import contextlib
import functools

import numpy as np

import concourse.bacc as bacc
import concourse.mybir as mybir
from concourse.bass_utils import run_bass_kernel_spmd
from concourse.masks import make_identity
from concourse.tile import TileContext

F32 = mybir.dt.float32
F16 = mybir.dt.float16

NCORES = 8
B, N, C = 64, 320, 768
H, DH = 12, 64
KS = C // 128  # 6 contraction subtiles
B_CORE = B // NCORES  # 8 batches per core
PAIR_TOK = 2 * N  # 640
NPAIR = B_CORE // 2  # 4
TOK_CORE = B_CORE * N  # 2560
L = 64  # template length
S = N - L  # search length 256

# key chunks of one batch's 320 tokens
KT_CHUNKS = [(0, 128), (128, 128), (256, 64)]

# es physical position within a quad of heads {4g..4g+3}: head 4g+r -> pos
QPOS = [0, 2, 1, 3]


def pos_of(h):
    return 4 * (h // 4) + QPOS[h % 4]


def tpos_of(h):
    """Template-es position: evens at 0..5, odds at 6..11."""
    return (h // 2) if h % 2 == 0 else 6 + h // 2


def _load_x(nc, pools, x_ap, p):
    x_nat = pools["x_nat"].tile([128, 5, C], F32, tag="x_nat")
    for t in range(5):
        r0 = p * PAIR_TOK + t * 128
        nc.sync.dma_start(x_nat[:, t, :], x_ap[r0 : r0 + 128, :])
    return x_nat


def _emit_xT(nc, pools, ident32, x_nat):
    """PE-transpose x pair -> x^T fp16 [128, KS, 640]."""
    big2 = pools["big2"]
    xT = pools["xT"].tile([128, KS, PAIR_TOK], F16, tag="xT")
    for fc in range(KS):
        ps = big2.tile([128, PAIR_TOK], F32, tag="big2")
        for t in range(5):
            nc.tensor.transpose(
                ps[:, t * 128 : (t + 1) * 128],
                x_nat[:, t, fc * 128 : (fc + 1) * 128],
                ident32,
            )
        if fc % 2 == 0:
            nc.scalar.copy(xT[:, fc, :], ps[:, :])
        else:
            nc.vector.tensor_copy(xT[:, fc, :], ps[:, :])
    return xT


def _emit_qk(nc, pools, wqkv16, xT):
    """q^T / k^T: features on partitions, heads (2fc, 2fc+1) stacked."""
    big2 = pools["big2"]
    qkT = pools["qkT"].tile([128, 2 * KS, PAIR_TOK], F16, tag="qkT")
    for fc in range(2 * KS):
        ps = big2.tile([128, 1024], F32, tag="big2")
        for ks in range(KS):
            lhsT = wqkv16[:, ks, fc * 128 : (fc + 1) * 128]
            nc.tensor.matmul(
                ps[:, 0:320],
                lhsT=lhsT,
                rhs=xT[:, ks, 0:320],
                start=(ks == 0),
                stop=(ks == KS - 1),
            )
            nc.tensor.matmul(
                ps[:, 512:832],
                lhsT=lhsT,
                rhs=xT[:, ks, 320:640],
                start=(ks == 0),
                stop=(ks == KS - 1),
            )
        nc.scalar.copy(
            qkT[:, fc, :].rearrange("p (b x) -> p b x", b=2),
            ps[:, :].rearrange("p (b x) -> p b x", b=2)[:, :, 0:320],
        )
    return qkT


def _emit_batch(nc, pools, wqkv16, xT, qkT, attnT, btok):
    """Scores + exp + V + transposed PV + normalize for one batch."""
    big2 = pools["big2"]
    pop = pools["po"]

    # ---- v natural [tok, h, 0:65]: ones column FIRST, v at 1:65 ----
    va = pools["va"].tile([128, 3, H, 66], F16, tag="va")
    for ci, (koff, ksz) in enumerate(KT_CHUNKS):
        ps = big2.tile([128, C], F32, tag="big2")
        for half, (n0, nw) in enumerate([(0, 512), (512, 256)]):
            for ks in range(KS):
                nc.tensor.matmul(
                    ps[:ksz, n0 : n0 + nw],
                    lhsT=xT[:, ks, btok + koff : btok + koff + ksz],
                    rhs=wqkv16[:, ks, 2 * C + n0 : 2 * C + n0 + nw],
                    start=(ks == 0),
                    stop=(ks == KS - 1),
                )
        if ci % 2 == 0:
            nc.vector.tensor_copy(
                va[:ksz, ci, :, 1:65],
                ps[:ksz, 0:768].rearrange("p (h d) -> p h d", d=64),
            )
        else:
            nc.scalar.copy(
                va[:ksz, ci, :, 1:65],
                ps[:ksz, 0:768].rearrange("p (h d) -> p h d", d=64),
            )
    nc.vector.memset(va[:, :, :, 0], 1.0)

    # ---- template scores first (esm is one cheap ACT op, needed by PV) ----
    # esm positions: evens 0..5, odds 6..11
    esm = pools["esm"].tile([64, H, L], F16, tag="esm")
    psm = big2.tile([128, 1024], F32, tag="big2")
    for j in range(KS):
        for par in range(2):
            h = 2 * j + par
            r0 = 64 * par
            dst0 = 512 * par + 64 * j
            nc.tensor.matmul(
                psm[0:64, dst0 : dst0 + 64],
                lhsT=qkT[r0 : r0 + 64, KS + j, btok : btok + L],
                rhs=qkT[r0 : r0 + 64, j, btok : btok + L],
                start=True,
                stop=True,
                tile_position=(r0, 0),
            )
    nc.scalar.activation(
        esm[:, :, :].rearrange("p (b h) q -> p b h q", b=2),
        psm[0:64, :]
        .rearrange("p (b x) -> p b x", b=2)[:, :, 0:384]
        .rearrange("p b (h q) -> p b h q", q=L),
        mybir.ActivationFunctionType.Exp,
        scale=0.125,
    )

    # ---- per head quad: search scores (row-paired) + exp, then PV ----
    # es layout [keys, ci, pos, 256] with pos = quad-permuted head order
    es = pools["es"].tile([128, 3, H, S], F16, tag="es")
    for g in range(3):
        for ci, (koff, ksz) in enumerate(KT_CHUNKS):
            ps = big2.tile([128, 1024], F32, tag="big2")
            for j in range(2):  # dual-issue slots within the quad
                for par in range(2):  # row-group parity
                    h = 4 * g + 2 * j + par
                    fc = h // 2
                    r0 = 64 * par
                    dst0 = QPOS[h % 4] * 256
                    nc.tensor.matmul(
                        ps[:ksz, dst0 : dst0 + 256],
                        lhsT=qkT[
                            r0 : r0 + 64, KS + fc, btok + koff : btok + koff + ksz
                        ],
                        rhs=qkT[r0 : r0 + 64, fc, btok + L : btok + N],
                        start=True,
                        stop=True,
                        tile_position=(r0, 0),
                    )
            nc.scalar.activation(
                es[:ksz, ci, 4 * g : 4 * g + 4, :],
                ps[:ksz, :].rearrange("p (h q) -> p h q", q=S),
                mybir.ActivationFunctionType.Exp,
                scale=0.125,
            )

        # ---- PV transposed + normalize for the 4 heads of this quad ----
        for h in range(4 * g, 4 * g + 4):
            fc = h // 2
            par = h % 2
            po = pop.tile([65, N], F32, tag="po")
            # template part: cols 0:64 (opens the accumulation group)
            nc.tensor.matmul(
                po[:, 0:L],
                lhsT=va[0:64, 0, h, 0:65],
                rhs=esm[:, tpos_of(h), :],
                start=True,
                stop=False,
            )
            # search part: cols 64:320, accumulated over key chunks
            for ci, (koff, ksz) in enumerate(KT_CHUNKS):
                nc.tensor.matmul(
                    po[:, L:N],
                    lhsT=va[0:ksz, ci, h, 0:65],
                    rhs=es[0:ksz, ci, pos_of(h), :],
                    start=False,
                    stop=(ci == 2),
                )
            # evacuate po quickly so the psum bank frees for the next head;
            # alternate engines so neither queue's latency gates the release
            poc = pools["poc"].tile([65, N], F32, tag="poc")
            if par == 0:
                nc.scalar.copy(poc[:, :], po[:, :])
            else:
                nc.vector.tensor_copy(poc[:, :], po[:, :])
            # denominator row 0: gpsimd-broadcast, fast reciprocal, multiply
            bc = pools["bc"].tile([65, N], F32, tag="bc")
            nc.gpsimd.partition_broadcast(bc[:, :], poc[0:1, :])
            rcpb = pools["rcpb"].tile([65, N], F32, tag="rcpb")
            nc.vector.reciprocal_approx_fast(rcpb[:, :], bc[:, :])
            tmp = pools["tmp"].tile([65, N], F16, tag="tmp")
            nc.vector.tensor_tensor(
                tmp[0:65, :], poc[0:65, :], rcpb[0:65, :], mybir.AluOpType.mult
            )
            nc.sync.dma_start(
                attnT[64 * par : 64 * par + 64, fc, btok : btok + N], tmp[1:65, :]
            )


def _emit_proj(nc, pools, wproj16, bias_bc, attnT, out_ap, p, chunks):
    """Output projection + bias for one pair (chunks of 128 tokens)."""
    big2 = pools["big2"]
    for tch in chunks:
        pp = big2.tile([128, C], F32, tag="big2")
        for half, (n0, nw) in enumerate([(0, 512), (512, 256)]):
            for ks in range(KS):
                nc.tensor.matmul(
                    pp[:, n0 : n0 + nw],
                    lhsT=attnT[:, ks, tch * 128 : (tch + 1) * 128],
                    rhs=wproj16[:, ks, n0 : n0 + nw],
                    start=(ks == 0),
                    stop=(ks == KS - 1),
                )
        out_sb = pools["out"].tile([128, C], F32, tag="out")
        nc.vector.tensor_tensor(
            out_sb[:, :], pp[:, :], bias_bc[:, :], mybir.AluOpType.add
        )
        row0 = p * PAIR_TOK + tch * 128
        nc.sync.dma_start(out_ap[row0 : row0 + 128, :], out_sb[:, :])


def build_kernel():
    nc = bacc.Bacc("TRN2", target_bir_lowering=False)
    x_t = nc.dram_tensor("x", [TOK_CORE, C], F32, kind="ExternalInput")
    wqkv_t = nc.dram_tensor("W_qkv", [C, 3 * C], F32, kind="ExternalInput")
    wproj_t = nc.dram_tensor("W_proj", [C, C], F32, kind="ExternalInput")
    bias_t = nc.dram_tensor("b_proj", [C], F32, kind="ExternalInput")
    out_t = nc.dram_tensor("out", [TOK_CORE, C], F32, kind="ExternalOutput")

    with TileContext(nc) as tc:
        with contextlib.ExitStack() as ctx:
            pools = {
                "const": ctx.enter_context(tc.tile_pool(name="const", bufs=1)),
                "stage": ctx.enter_context(tc.tile_pool(name="stage", bufs=2)),
                "x_nat": ctx.enter_context(tc.tile_pool(name="x_nat", bufs=1)),
                "xT": ctx.enter_context(tc.tile_pool(name="xT", bufs=2)),
                "qkT": ctx.enter_context(tc.tile_pool(name="qkT", bufs=2)),
                "va": ctx.enter_context(tc.tile_pool(name="va", bufs=2)),
                "es": ctx.enter_context(tc.tile_pool(name="es", bufs=2)),
                "esm": ctx.enter_context(tc.tile_pool(name="esm", bufs=2)),
                "attnT": ctx.enter_context(tc.tile_pool(name="attnT", bufs=2)),
                "out": ctx.enter_context(tc.tile_pool(name="out", bufs=3)),
                "poc": ctx.enter_context(tc.tile_pool(name="poc", bufs=5)),
                "bc": ctx.enter_context(tc.tile_pool(name="bc", bufs=5)),
                "tmp": ctx.enter_context(tc.tile_pool(name="tmp", bufs=6)),
                "rcpb": ctx.enter_context(tc.tile_pool(name="rcpb", bufs=4)),
                "big2": ctx.enter_context(
                    tc.tile_pool(name="big2", bufs=2, space="PSUM")
                ),
                "po": ctx.enter_context(
                    tc.tile_pool(name="po", bufs=4, space="PSUM")
                ),
            }
            const = pools["const"]

            wqkv16 = const.tile([128, KS, 3 * C], F16, tag="wqkv16")
            wproj16 = const.tile([128, KS, C], F16, tag="wproj16")
            bias_bc = const.tile([128, C], F32, tag="bias_bc")
            ident32 = const.tile([128, 128], F32, tag="ident32")
            make_identity(nc, ident32)
            x0 = _load_x(nc, pools, x_t.ap(), 0)

            # stage W_qkv by 128-column blocks so the first qk matmul only
            # waits for one ~1.2MB block instead of the whole 7MB weight
            for fb in range(18):
                st = pools["stage"].tile([128, KS, 128], F32, tag="stage")
                nc.sync.dma_start(
                    st[:],
                    wqkv_t.ap()[:, fb * 128 : (fb + 1) * 128].rearrange(
                        "(k p) f -> p k f", p=128
                    ),
                )
                if fb % 2 == 0:
                    nc.scalar.copy(
                        wqkv16[:, :, fb * 128 : (fb + 1) * 128], st[:]
                    )
                else:
                    nc.vector.tensor_copy(
                        wqkv16[:, :, fb * 128 : (fb + 1) * 128], st[:]
                    )
            # software pipeline: transposes of pair p+1 are emitted between
            # batch 0 and batch 1 of pair p; wproj/bias staging is emitted
            # after the first batch so it does not crowd the startup
            xT_next = _emit_xT(nc, pools, ident32, x0)
            for p in range(NPAIR):
                xT = xT_next
                qkT = _emit_qk(nc, pools, wqkv16, xT)
                attnT = pools["attnT"].tile([128, KS, PAIR_TOK], F16, tag="attnT")
                _emit_batch(nc, pools, wqkv16, xT, qkT, attnT, 0)
                if p == 0:
                    for ks in range(KS):
                        stp = pools["stage"].tile([128, C], F32, tag="stagep")
                        nc.sync.dma_start(
                            stp[:], wproj_t.ap()[ks * 128 : (ks + 1) * 128, :]
                        )
                        if ks % 2 == 0:
                            nc.scalar.copy(wproj16[:, ks, :], stp[:])
                        else:
                            nc.vector.tensor_copy(wproj16[:, ks, :], stp[:])
                    brow = pools["stage"].tile([1, C], F32, tag="stagep")
                    nc.sync.dma_start(brow[0:1, :], bias_t.ap().unsqueeze(0))
                    nc.gpsimd.partition_broadcast(bias_bc[:, :], brow[0:1, :])
                if p + 1 < NPAIR:
                    xn = _load_x(nc, pools, x_t.ap(), p + 1)
                    xT_next = _emit_xT(nc, pools, ident32, xn)
                _emit_batch(nc, pools, wqkv16, xT, qkT, attnT, N)
                _emit_proj(
                    nc, pools, wproj16, bias_bc, attnT, out_t.ap(), p,
                    (0, 1, 2, 3, 4),
                )

    nc.compile()
    return nc


@functools.cache
def _get_nc():
    return build_kernel()


def kernel(**inputs):
    x = np.ascontiguousarray(np.asarray(inputs["x"], dtype=np.float32))
    wqkv = np.ascontiguousarray(np.asarray(inputs["W_qkv"], dtype=np.float32))
    wproj = np.ascontiguousarray(np.asarray(inputs["W_proj"], dtype=np.float32))
    bias = np.ascontiguousarray(np.asarray(inputs["b_proj"], dtype=np.float32))
    t_h = int(inputs.get("t_h", 8))
    t_w = int(inputs.get("t_w", 8))
    assert t_h * t_w == 64, "kernel built for template length 64"
    assert x.shape == (B, N, C)

    nc = _get_nc()
    in_maps = [
        {
            "x": x[c * B_CORE : (c + 1) * B_CORE].reshape(TOK_CORE, C),
            "W_qkv": wqkv,
            "W_proj": wproj,
            "b_proj": bias,
        }
        for c in range(NCORES)
    ]
    res = run_bass_kernel_spmd(nc, in_maps, core_ids=list(range(NCORES)))
    out = np.concatenate(
        [r["out"].reshape(B_CORE, N, C) for r in res.results], axis=0
    )
    return out.astype(np.float32)


if __name__ == "__main__":
    _get_nc()
    print("kernel built OK")


# revision 31
# speedup vs baseline: 1.0205x; 1.0034x over previous
"""Trainium2 Bass kernel for MixformerAttention (sparse attention), v3.

Problem shape (hardcoded from the problem spec):
  x [B=64, N=320, C=768], W_qkv [768, 2304], W_proj [768, 768], b_proj [768]
  H=12 heads, Dh=64, template length L = t_h*t_w = 64, search = 256.

Sharding: data-parallel over batch across 8 NeuronCores (8 batches/core).

Per-core pipeline (batches processed in pairs of 2 -> 640 tokens = 5x128):
  1. DMA x pair -> SBUF, PE-transpose to x^T (C on partitions), cast fp16.
     The next pair's load+transposes are emitted mid-pair (software pipeline)
     so the PE never sits in a long transpose-only or DMA-wait stretch.
  2. q^T,k^T matmuls (features on partitions); per chunk fc the heads
     (2fc, 2fc+1) sit stacked on partitions 0:64 / 64:128.
     V natural matmul -> v [tok, head, 1+64] with a LEADING ones column.
  3. Scores S^T = k q^T per head, ROW-PAIRED: even head on PE row-group 0,
     odd head on row-group 64 (explicit tile_position) -> 2x concurrency.
     exp via ACT (scale=1/8) -> es fp16 (head order permuted per quad).
  4. PV transposed: O^T[d,q] = [1|v]^T @ expS^T per head -> psum [65, 320]
     (template cols 0:64, search 64:320); row 0 = softmax denominator.
     ACT-copy po -> SBUF (releases the psum bank fast), gpsimd-broadcast
     the denominator row, DVE reciprocal_approx_fast, DVE multiply, then
     DMA partition-shift rows 1:65 into attn^T rows 0:64 / 64:128.
  5. attn^T is directly the proj lhsT: out = attn^T.T @ W_proj + bias,
     5x128-token chunks per pair -> DMA out fp32.

All matmuls fp16 operands (1 cycle/row on PE, fp32 PSUM accumulate).
"""

import contextlib
import functools

import numpy as np

import concourse.bacc as bacc
import concourse.mybir as mybir
from concourse.bass_utils import run_bass_kernel_spmd
from concourse.masks import make_identity
from concourse.tile import TileContext

F32 = mybir.dt.float32
F16 = mybir.dt.float16

NCORES = 8
B, N, C = 64, 320, 768
H, DH = 12, 64
KS = C // 128  # 6 contraction subtiles
B_CORE = B // NCORES  # 8 batches per core
PAIR_TOK = 2 * N  # 640
NPAIR = B_CORE // 2  # 4
TOK_CORE = B_CORE * N  # 2560
L = 64  # template length
S = N - L  # search length 256

# key chunks of one batch's 320 tokens
KT_CHUNKS = [(0, 128), (128, 128), (256, 64)]

# es physical position within a quad of heads {4g..4g+3}: head 4g+r -> pos
QPOS = [0, 2, 1, 3]


def pos_of(h):
    return 4 * (h // 4) + QPOS[h % 4]


def tpos_of(h):
    """Template-es position: evens at 0..5, odds at 6..11."""
    return (h // 2) if h % 2 == 0 else 6 + h // 2


def _load_x(nc, pools, x_ap, p):
    """DMA x per 128-token chunk, cast to fp16 immediately (small staging)."""
    x16 = pools["x16"].tile([128, 5, C], F16, tag="x16")
    for t in range(5):
        r0 = p * PAIR_TOK + t * 128
        xst = pools["xst"].tile([128, C], F32, tag="xst")
        nc.sync.dma_start(xst[:, :], x_ap[r0 : r0 + 128, :])
        if t % 2 == 0:
            nc.vector.tensor_copy(x16[:, t, :], xst[:, :])
        else:
            nc.scalar.copy(x16[:, t, :], xst[:, :])
    return x16


def _emit_xT(nc, pools, ident16, x16):
    """Transpose x pair -> x^T fp16 via regular matmuls against identity.

    Regular matmuls (unlike transpose-mode ones) count as PE-busy for the
    HAM clock gate, so the transpose stretch does not re-throttle the PE.
    """
    big2 = pools["big2"]
    xT = pools["xT"].tile([128, KS, PAIR_TOK], F16, tag="xT")
    for fc in range(KS):
        ps = big2.tile([128, PAIR_TOK], F32, tag="big2")
        for t in range(5):
            nc.tensor.matmul(
                ps[:, t * 128 : (t + 1) * 128],
                lhsT=x16[:, t, fc * 128 : (fc + 1) * 128],
                rhs=ident16,
                start=True,
                stop=True,
            )
        if fc % 2 == 0:
            nc.scalar.copy(xT[:, fc, :], ps[:, :])
        else:
            nc.vector.tensor_copy(xT[:, fc, :], ps[:, :])
    return xT


def _emit_qk(nc, pools, wqkv16, xT):
    """q^T / k^T: features on partitions, heads (2fc, 2fc+1) stacked."""
    big2 = pools["big2"]
    qkT = pools["qkT"].tile([128, 2 * KS, PAIR_TOK], F16, tag="qkT")
    for fc in range(2 * KS):
        ps = big2.tile([128, 1024], F32, tag="big2")
        for ks in range(KS):
            lhsT = wqkv16[:, ks, fc * 128 : (fc + 1) * 128]
            nc.tensor.matmul(
                ps[:, 0:320],
                lhsT=lhsT,
                rhs=xT[:, ks, 0:320],
                start=(ks == 0),
                stop=(ks == KS - 1),
            )
            nc.tensor.matmul(
                ps[:, 512:832],
                lhsT=lhsT,
                rhs=xT[:, ks, 320:640],
                start=(ks == 0),
                stop=(ks == KS - 1),
            )
        nc.scalar.copy(
            qkT[:, fc, :].rearrange("p (b x) -> p b x", b=2),
            ps[:, :].rearrange("p (b x) -> p b x", b=2)[:, :, 0:320],
        )
    return qkT


def _emit_batch(nc, pools, wqkv16, xT, qkT, attnT, btok):
    """Scores + exp + V + transposed PV + normalize for one batch."""
    big2 = pools["big2"]
    pop = pools["po"]

    # ---- v natural [tok, h, 0:65]: ones column FIRST, v at 1:65 ----
    va = pools["va"].tile([128, 3, H, 66], F16, tag="va")
    for ci, (koff, ksz) in enumerate(KT_CHUNKS):
        ps = big2.tile([128, C], F32, tag="big2")
        for half, (n0, nw) in enumerate([(0, 512), (512, 256)]):
            for ks in range(KS):
                nc.tensor.matmul(
                    ps[:ksz, n0 : n0 + nw],
                    lhsT=xT[:, ks, btok + koff : btok + koff + ksz],
                    rhs=wqkv16[:, ks, 2 * C + n0 : 2 * C + n0 + nw],
                    start=(ks == 0),
                    stop=(ks == KS - 1),
                )
        if ci % 2 == 0:
            nc.vector.tensor_copy(
                va[:ksz, ci, :, 1:65],
                ps[:ksz, 0:768].rearrange("p (h d) -> p h d", d=64),
            )
        else:
            nc.scalar.copy(
                va[:ksz, ci, :, 1:65],
                ps[:ksz, 0:768].rearrange("p (h d) -> p h d", d=64),
            )
    nc.vector.memset(va[:, :, :, 0], 1.0)

    # ---- template scores first (esm is one cheap ACT op, needed by PV) ----
    # esm positions: evens 0..5, odds 6..11
    esm = pools["esm"].tile([64, H, L], F16, tag="esm")
    psm = big2.tile([128, 1024], F32, tag="big2")
    for j in range(KS):
        for par in range(2):
            h = 2 * j + par
            r0 = 64 * par
            dst0 = 512 * par + 64 * j
            nc.tensor.matmul(
                psm[0:64, dst0 : dst0 + 64],
                lhsT=qkT[r0 : r0 + 64, KS + j, btok : btok + L],
                rhs=qkT[r0 : r0 + 64, j, btok : btok + L],
                start=True,
                stop=True,
                tile_position=(r0, 0),
            )
    nc.scalar.activation(
        esm[:, :, :].rearrange("p (b h) q -> p b h q", b=2),
        psm[0:64, :]
        .rearrange("p (b x) -> p b x", b=2)[:, :, 0:384]
        .rearrange("p b (h q) -> p b h q", q=L),
        mybir.ActivationFunctionType.Exp,
        scale=0.125,
    )

    # ---- per head quad: search scores (row-paired) + exp, then PV ----
    # es layout [keys, ci, pos, 256] with pos = quad-permuted head order
    es = pools["es"].tile([128, 3, H, S], F16, tag="es")
    for g in range(3):
        for ci, (koff, ksz) in enumerate(KT_CHUNKS):
            ps = big2.tile([128, 1024], F32, tag="big2")
            for j in range(2):  # dual-issue slots within the quad
                for par in range(2):  # row-group parity
                    h = 4 * g + 2 * j + par
                    fc = h // 2
                    r0 = 64 * par
                    dst0 = QPOS[h % 4] * 256
                    nc.tensor.matmul(
                        ps[:ksz, dst0 : dst0 + 256],
                        lhsT=qkT[
                            r0 : r0 + 64, KS + fc, btok + koff : btok + koff + ksz
                        ],
                        rhs=qkT[r0 : r0 + 64, fc, btok + L : btok + N],
                        start=True,
                        stop=True,
                        tile_position=(r0, 0),
                    )
            nc.scalar.activation(
                es[:ksz, ci, 4 * g : 4 * g + 4, :],
                ps[:ksz, :].rearrange("p (h q) -> p h q", q=S),
                mybir.ActivationFunctionType.Exp,
                scale=0.125,
            )

        # ---- PV transposed + normalize for the 4 heads of this quad ----
        for h in range(4 * g, 4 * g + 4):
            fc = h // 2
            par = h % 2
            po = pop.tile([65, N], F32, tag="po")
            # template part: cols 0:64 (opens the accumulation group)
            nc.tensor.matmul(
                po[:, 0:L],
                lhsT=va[0:64, 0, h, 0:65],
                rhs=esm[:, tpos_of(h), :],
                start=True,
                stop=False,
            )
            # search part: cols 64:320, accumulated over key chunks
            for ci, (koff, ksz) in enumerate(KT_CHUNKS):
                nc.tensor.matmul(
                    po[:, L:N],
                    lhsT=va[0:ksz, ci, h, 0:65],
                    rhs=es[0:ksz, ci, pos_of(h), :],
                    start=False,
                    stop=(ci == 2),
                )
            # evacuate po quickly so the psum bank frees for the next head;
            # alternate engines so neither queue's latency gates the release
            poc = pools["poc"].tile([65, N], F32, tag="poc")
            if par == 0:
                nc.scalar.copy(poc[:, :], po[:, :])
            else:
                nc.vector.tensor_copy(poc[:, :], po[:, :])
            # denominator row 0: gpsimd-broadcast, fast reciprocal, multiply
            bc = pools["bc"].tile([65, N], F32, tag="bc")
            nc.gpsimd.partition_broadcast(bc[:, :], poc[0:1, :])
            rcpb = pools["rcpb"].tile([65, N], F32, tag="rcpb")
            nc.vector.reciprocal_approx_fast(rcpb[:, :], bc[:, :])
            tmp = pools["tmp"].tile([65, N], F16, tag="tmp")
            nc.vector.tensor_tensor(
                tmp[0:65, :], poc[0:65, :], rcpb[0:65, :], mybir.AluOpType.mult
            )
            nc.sync.dma_start(
                attnT[64 * par : 64 * par + 64, fc, btok : btok + N], tmp[1:65, :]
            )


def _emit_proj(nc, pools, wproj16, bias_bc, attnT, out_ap, p, chunks):
    """Output projection + bias for one pair (chunks of 128 tokens)."""
    big2 = pools["big2"]
    for tch in chunks:
        pp = big2.tile([128, C], F32, tag="big2")
        for half, (n0, nw) in enumerate([(0, 512), (512, 256)]):
            for ks in range(KS):
                nc.tensor.matmul(
                    pp[:, n0 : n0 + nw],
                    lhsT=attnT[:, ks, tch * 128 : (tch + 1) * 128],
                    rhs=wproj16[:, ks, n0 : n0 + nw],
                    start=(ks == 0),
                    stop=(ks == KS - 1),
                )
        out_sb = pools["out"].tile([128, C], F32, tag="out")
        nc.vector.tensor_tensor(
            out_sb[:, :], pp[:, :], bias_bc[:, :], mybir.AluOpType.add
        )
        row0 = p * PAIR_TOK + tch * 128
        nc.sync.dma_start(out_ap[row0 : row0 + 128, :], out_sb[:, :])


def build_kernel():
    nc = bacc.Bacc("TRN2", target_bir_lowering=False)
    x_t = nc.dram_tensor("x", [TOK_CORE, C], F32, kind="ExternalInput")
    wqkv_t = nc.dram_tensor("W_qkv", [C, 3 * C], F32, kind="ExternalInput")
    wproj_t = nc.dram_tensor("W_proj", [C, C], F32, kind="ExternalInput")
    bias_t = nc.dram_tensor("b_proj", [C], F32, kind="ExternalInput")
    out_t = nc.dram_tensor("out", [TOK_CORE, C], F32, kind="ExternalOutput")

    with TileContext(nc) as tc:
        with contextlib.ExitStack() as ctx:
            pools = {
                "const": ctx.enter_context(tc.tile_pool(name="const", bufs=1)),
                "stage": ctx.enter_context(tc.tile_pool(name="stage", bufs=2)),
                "xst": ctx.enter_context(tc.tile_pool(name="xst", bufs=2)),
                "xT": ctx.enter_context(tc.tile_pool(name="xT", bufs=2)),
                "x16": ctx.enter_context(tc.tile_pool(name="x16", bufs=1)),
                "qkT": ctx.enter_context(tc.tile_pool(name="qkT", bufs=2)),
                "va": ctx.enter_context(tc.tile_pool(name="va", bufs=2)),
                "es": ctx.enter_context(tc.tile_pool(name="es", bufs=2)),
                "esm": ctx.enter_context(tc.tile_pool(name="esm", bufs=2)),
                "attnT": ctx.enter_context(tc.tile_pool(name="attnT", bufs=2)),
                "out": ctx.enter_context(tc.tile_pool(name="out", bufs=3)),
                "poc": ctx.enter_context(tc.tile_pool(name="poc", bufs=5)),
                "bc": ctx.enter_context(tc.tile_pool(name="bc", bufs=5)),
                "tmp": ctx.enter_context(tc.tile_pool(name="tmp", bufs=6)),
                "rcpb": ctx.enter_context(tc.tile_pool(name="rcpb", bufs=4)),
                "big2": ctx.enter_context(
                    tc.tile_pool(name="big2", bufs=2, space="PSUM")
                ),
                "po": ctx.enter_context(
                    tc.tile_pool(name="po", bufs=4, space="PSUM")
                ),
            }
            const = pools["const"]

            wqkv16 = const.tile([128, KS, 3 * C], F16, tag="wqkv16")
            wproj16 = const.tile([128, KS, C], F16, tag="wproj16")
            bias_bc = const.tile([128, C], F32, tag="bias_bc")
            ident32 = const.tile([128, 128], F32, tag="ident32")
            make_identity(nc, ident32)
            ident16 = const.tile([128, 128], F16, tag="ident16")
            make_identity(nc, ident16)
            x0 = _load_x(nc, pools, x_t.ap(), 0)

            # stage W_qkv by 128-column blocks so the first qk matmul only
            # waits for one ~1.2MB block instead of the whole 7MB weight
            for fb in range(18):
                st = pools["stage"].tile([128, KS, 128], F32, tag="stage")
                nc.sync.dma_start(
                    st[:],
                    wqkv_t.ap()[:, fb * 128 : (fb + 1) * 128].rearrange(
                        "(k p) f -> p k f", p=128
                    ),
                )
                if fb % 2 == 0:
                    nc.scalar.copy(
                        wqkv16[:, :, fb * 128 : (fb + 1) * 128], st[:]
                    )
                else:
                    nc.vector.tensor_copy(
                        wqkv16[:, :, fb * 128 : (fb + 1) * 128], st[:]
                    )
            # software pipeline: transposes of pair p+1 are emitted between
            # batch 0 and batch 1 of pair p; wproj/bias staging is emitted
            # after the first batch so it does not crowd the startup
            xT_next = _emit_xT(nc, pools, ident16, x0)
            for p in range(NPAIR):
                xT = xT_next
                qkT = _emit_qk(nc, pools, wqkv16, xT)
                attnT = pools["attnT"].tile([128, KS, PAIR_TOK], F16, tag="attnT")
                _emit_batch(nc, pools, wqkv16, xT, qkT, attnT, 0)
                if p == 0:
                    for ks in range(KS):
                        stp = pools["stage"].tile([128, C], F32, tag="stagep")
                        nc.sync.dma_start(
                            stp[:], wproj_t.ap()[ks * 128 : (ks + 1) * 128, :]
                        )
                        if ks % 2 == 0:
                            nc.scalar.copy(wproj16[:, ks, :], stp[:])
                        else:
                            nc.vector.tensor_copy(wproj16[:, ks, :], stp[:])
                    brow = pools["stage"].tile([1, C], F32, tag="stagep")
                    nc.sync.dma_start(brow[0:1, :], bias_t.ap().unsqueeze(0))
                    nc.gpsimd.partition_broadcast(bias_bc[:, :], brow[0:1, :])
                if p + 1 < NPAIR:
                    xn = _load_x(nc, pools, x_t.ap(), p + 1)
                    xT_next = _emit_xT(nc, pools, ident16, xn)
                _emit_batch(nc, pools, wqkv16, xT, qkT, attnT, N)
                _emit_proj(
                    nc, pools, wproj16, bias_bc, attnT, out_t.ap(), p,
                    (0, 1, 2, 3, 4),
                )

    nc.compile()
    return nc


@functools.cache
def _get_nc():
    return build_kernel()


def kernel(**inputs):
    x = np.ascontiguousarray(np.asarray(inputs["x"], dtype=np.float32))
    wqkv = np.ascontiguousarray(np.asarray(inputs["W_qkv"], dtype=np.float32))
    wproj = np.ascontiguousarray(np.asarray(inputs["W_proj"], dtype=np.float32))
    bias = np.ascontiguousarray(np.asarray(inputs["b_proj"], dtype=np.float32))
    t_h = int(inputs.get("t_h", 8))
    t_w = int(inputs.get("t_w", 8))
    assert t_h * t_w == 64, "kernel built for template length 64"
    assert x.shape == (B, N, C)

    nc = _get_nc()
    in_maps = [
        {
            "x": x[c * B_CORE : (c + 1) * B_CORE].reshape(TOK_CORE, C),
            "W_qkv": wqkv,
            "W_proj": wproj,
            "b_proj": bias,
        }
        for c in range(NCORES)
    ]
    res = run_bass_kernel_spmd(nc, in_maps, core_ids=list(range(NCORES)))
    out = np.concatenate(
        [r["out"].reshape(B_CORE, N, C) for r in res.results], axis=0
    )
    return out.astype(np.float32)


if __name__ == "__main__":
    _get_nc()
    print("kernel built OK")


# revision 32
# speedup vs baseline: 1.0486x; 1.0275x over previous
"""Trainium2 Bass kernel for MixformerAttention (sparse attention), v3.

Problem shape (hardcoded from the problem spec):
  x [B=64, N=320, C=768], W_qkv [768, 2304], W_proj [768, 768], b_proj [768]
  H=12 heads, Dh=64, template length L = t_h*t_w = 64, search = 256.

Sharding: data-parallel over batch across 8 NeuronCores (8 batches/core).

Per-core pipeline (batches processed in pairs of 2 -> 640 tokens = 5x128):
  1. DMA x pair -> SBUF, PE-transpose to x^T (C on partitions), cast fp16.
     The next pair's load+transposes are emitted mid-pair (software pipeline)
     so the PE never sits in a long transpose-only or DMA-wait stretch.
  2. q^T,k^T matmuls (features on partitions); per chunk fc the heads
     (2fc, 2fc+1) sit stacked on partitions 0:64 / 64:128.
     V natural matmul -> v [tok, head, 1+64] with a LEADING ones column.
  3. Scores S^T = k q^T per head, ROW-PAIRED: even head on PE row-group 0,
     odd head on row-group 64 (explicit tile_position) -> 2x concurrency.
     exp via ACT (scale=1/8) -> es fp16 (head order permuted per quad).
  4. PV transposed: O^T[d,q] = [1|v]^T @ expS^T per head -> psum [65, 320]
     (template cols 0:64, search 64:320); row 0 = softmax denominator.
     ACT-copy po -> SBUF (releases the psum bank fast), gpsimd-broadcast
     the denominator row, DVE reciprocal_approx_fast, DVE multiply, then
     DMA partition-shift rows 1:65 into attn^T rows 0:64 / 64:128.
  5. attn^T is directly the proj lhsT: out = attn^T.T @ W_proj + bias,
     5x128-token chunks per pair -> DMA out fp32.

All matmuls fp16 operands (1 cycle/row on PE, fp32 PSUM accumulate).
"""

import contextlib
import functools

import numpy as np

import concourse.bacc as bacc
import concourse.mybir as mybir
from concourse.bass_utils import run_bass_kernel_spmd
from concourse.masks import make_identity
from concourse.tile import TileContext

F32 = mybir.dt.float32
F16 = mybir.dt.float16

NCORES = 8
B, N, C = 64, 320, 768
H, DH = 12, 64
KS = C // 128  # 6 contraction subtiles
B_CORE = B // NCORES  # 8 batches per core
PAIR_TOK = 2 * N  # 640
NPAIR = B_CORE // 2  # 4
TOK_CORE = B_CORE * N  # 2560
L = 64  # template length
S = N - L  # search length 256

# key chunks of one batch's 320 tokens
KT_CHUNKS = [(0, 128), (128, 128), (256, 64)]

# es physical position within a quad of heads {4g..4g+3}: head 4g+r -> pos
QPOS = [0, 2, 1, 3]


def pos_of(h):
    return 4 * (h // 4) + QPOS[h % 4]


def tpos_of(h):
    """Template-es position: evens at 0..5, odds at 6..11."""
    return (h // 2) if h % 2 == 0 else 6 + h // 2


def _load_x(nc, pools, x_ap, p):
    """DMA x per 128-token chunk, cast to fp16 immediately (small staging)."""
    x16 = pools["x16"].tile([128, 5, C], F16, tag="x16")
    for t in range(5):
        r0 = p * PAIR_TOK + t * 128
        xst = pools["xst"].tile([128, C], F32, tag="xst")
        nc.sync.dma_start(xst[:, :], x_ap[r0 : r0 + 128, :])
        if t % 2 == 0:
            nc.vector.tensor_copy(x16[:, t, :], xst[:, :])
        else:
            nc.scalar.copy(x16[:, t, :], xst[:, :])
    return x16


def _emit_xT(nc, pools, ident16, x16):
    """Transpose x pair -> x^T fp16 via regular matmuls against identity.

    Regular matmuls (unlike transpose-mode ones) count as PE-busy for the
    HAM clock gate, so the transpose stretch does not re-throttle the PE.
    """
    big2 = pools["big2"]
    xT = pools["xT"].tile([128, KS, PAIR_TOK], F16, tag="xT")
    for fc in range(KS):
        ps = big2.tile([128, PAIR_TOK], F32, tag="big2")
        for t in range(5):
            nc.tensor.matmul(
                ps[:, t * 128 : (t + 1) * 128],
                lhsT=x16[:, t, fc * 128 : (fc + 1) * 128],
                rhs=ident16,
                start=True,
                stop=True,
            )
        if fc % 2 == 0:
            nc.scalar.copy(xT[:, fc, :], ps[:, :])
        else:
            nc.vector.tensor_copy(xT[:, fc, :], ps[:, :])
    return xT


def _emit_qk(nc, pools, wqkv16, xT):
    """q^T / k^T: features on partitions, heads (2fc, 2fc+1) stacked."""
    big2 = pools["big2"]
    qkT = pools["qkT"].tile([128, 2 * KS, PAIR_TOK], F16, tag="qkT")
    for fc in range(2 * KS):
        ps = big2.tile([128, 1024], F32, tag="big2")
        for ks in range(KS):
            lhsT = wqkv16[:, ks, fc * 128 : (fc + 1) * 128]
            nc.tensor.matmul(
                ps[:, 0:320],
                lhsT=lhsT,
                rhs=xT[:, ks, 0:320],
                start=(ks == 0),
                stop=(ks == KS - 1),
            )
            nc.tensor.matmul(
                ps[:, 512:832],
                lhsT=lhsT,
                rhs=xT[:, ks, 320:640],
                start=(ks == 0),
                stop=(ks == KS - 1),
            )
        nc.scalar.copy(
            qkT[:, fc, :].rearrange("p (b x) -> p b x", b=2),
            ps[:, :].rearrange("p (b x) -> p b x", b=2)[:, :, 0:320],
        )
    return qkT


def _emit_batch(nc, pools, wqkv16, xT, qkT, attnT, btok):
    """Scores + exp + V + transposed PV + normalize for one batch."""
    big2 = pools["big2"]
    pop = pools["po"]

    # ---- v natural [tok, h, 0:65]: ones column FIRST, v at 1:65 ----
    va = pools["va"].tile([128, 3, H, 66], F16, tag="va")
    for ci, (koff, ksz) in enumerate(KT_CHUNKS):
        ps = big2.tile([128, C], F32, tag="big2")
        for half, (n0, nw) in enumerate([(0, 512), (512, 256)]):
            for ks in range(KS):
                nc.tensor.matmul(
                    ps[:ksz, n0 : n0 + nw],
                    lhsT=xT[:, ks, btok + koff : btok + koff + ksz],
                    rhs=wqkv16[:, ks, 2 * C + n0 : 2 * C + n0 + nw],
                    start=(ks == 0),
                    stop=(ks == KS - 1),
                )
        if ci % 2 == 0:
            nc.vector.tensor_copy(
                va[:ksz, ci, :, 1:65],
                ps[:ksz, 0:768].rearrange("p (h d) -> p h d", d=64),
            )
        else:
            nc.scalar.copy(
                va[:ksz, ci, :, 1:65],
                ps[:ksz, 0:768].rearrange("p (h d) -> p h d", d=64),
            )
    nc.vector.memset(va[:, :, :, 0], 1.0)

    # ---- template scores first (esm is one cheap ACT op, needed by PV) ----
    # esm positions: evens 0..5, odds 6..11
    esm = pools["esm"].tile([64, H, L], F16, tag="esm")
    psm = big2.tile([128, 1024], F32, tag="big2")
    for j in range(KS):
        for par in range(2):
            h = 2 * j + par
            r0 = 64 * par
            dst0 = 512 * par + 64 * j
            nc.tensor.matmul(
                psm[0:64, dst0 : dst0 + 64],
                lhsT=qkT[r0 : r0 + 64, KS + j, btok : btok + L],
                rhs=qkT[r0 : r0 + 64, j, btok : btok + L],
                start=True,
                stop=True,
                tile_position=(r0, 0),
            )
    nc.scalar.activation(
        esm[:, :, :].rearrange("p (b h) q -> p b h q", b=2),
        psm[0:64, :]
        .rearrange("p (b x) -> p b x", b=2)[:, :, 0:384]
        .rearrange("p b (h q) -> p b h q", q=L),
        mybir.ActivationFunctionType.Exp,
        scale=0.125,
    )

    # ---- per head quad: search scores (row-paired) + exp, then PV ----
    # es layout [keys, ci, pos, 256] with pos = quad-permuted head order
    es = pools["es"].tile([128, 3, H, S], F16, tag="es")
    for g in range(3):
        for ci, (koff, ksz) in enumerate(KT_CHUNKS):
            ps = big2.tile([128, 1024], F32, tag="big2")
            for j in range(2):  # dual-issue slots within the quad
                for par in range(2):  # row-group parity
                    h = 4 * g + 2 * j + par
                    fc = h // 2
                    r0 = 64 * par
                    dst0 = QPOS[h % 4] * 256
                    nc.tensor.matmul(
                        ps[:ksz, dst0 : dst0 + 256],
                        lhsT=qkT[
                            r0 : r0 + 64, KS + fc, btok + koff : btok + koff + ksz
                        ],
                        rhs=qkT[r0 : r0 + 64, fc, btok + L : btok + N],
                        start=True,
                        stop=True,
                        tile_position=(r0, 0),
                    )
            nc.scalar.activation(
                es[:ksz, ci, 4 * g : 4 * g + 4, :],
                ps[:ksz, :].rearrange("p (h q) -> p h q", q=S),
                mybir.ActivationFunctionType.Exp,
                scale=0.125,
            )

        # ---- PV transposed + normalize for the 4 heads of this quad ----
        for h in range(4 * g, 4 * g + 4):
            fc = h // 2
            par = h % 2
            po = pop.tile([65, N], F32, tag="po")
            # template part: cols 0:64 (opens the accumulation group)
            nc.tensor.matmul(
                po[:, 0:L],
                lhsT=va[0:64, 0, h, 0:65],
                rhs=esm[:, tpos_of(h), :],
                start=True,
                stop=False,
            )
            # search part: cols 64:320, accumulated over key chunks
            for ci, (koff, ksz) in enumerate(KT_CHUNKS):
                nc.tensor.matmul(
                    po[:, L:N],
                    lhsT=va[0:ksz, ci, h, 0:65],
                    rhs=es[0:ksz, ci, pos_of(h), :],
                    start=False,
                    stop=(ci == 2),
                )
            # evacuate po quickly so the psum bank frees for the next head;
            # alternate engines so neither queue's latency gates the release
            poc = pools["poc"].tile([65, N], F32, tag="poc")
            if par == 0:
                nc.scalar.copy(poc[:, :], po[:, :])
            else:
                nc.vector.tensor_copy(poc[:, :], po[:, :])
            # denominator row 0: gpsimd-broadcast, fast reciprocal, multiply
            bc = pools["bc"].tile([65, N], F32, tag="bc")
            nc.gpsimd.partition_broadcast(bc[:, :], poc[0:1, :])
            rcpb = pools["rcpb"].tile([65, N], F32, tag="rcpb")
            nc.vector.reciprocal_approx_fast(rcpb[:, :], bc[:, :])
            tmp = pools["tmp"].tile([65, N], F16, tag="tmp")
            nc.vector.tensor_tensor(
                tmp[0:65, :], poc[0:65, :], rcpb[0:65, :], mybir.AluOpType.mult
            )
            nc.sync.dma_start(
                attnT[64 * par : 64 * par + 64, fc, btok : btok + N], tmp[1:65, :]
            )


def _emit_proj(nc, pools, wproj16, bias_bc, attnT, out_ap, p, chunks):
    """Output projection + bias for one pair (chunks of 128 tokens)."""
    big2 = pools["big2"]
    for tch in chunks:
        pp = big2.tile([128, C], F32, tag="big2")
        for half, (n0, nw) in enumerate([(0, 512), (512, 256)]):
            for ks in range(KS):
                nc.tensor.matmul(
                    pp[:, n0 : n0 + nw],
                    lhsT=attnT[:, ks, tch * 128 : (tch + 1) * 128],
                    rhs=wproj16[:, ks, n0 : n0 + nw],
                    start=(ks == 0),
                    stop=(ks == KS - 1),
                )
        out_sb = pools["out"].tile([128, C], F32, tag="out")
        nc.vector.tensor_tensor(
            out_sb[:, :], pp[:, :], bias_bc[:, :], mybir.AluOpType.add
        )
        row0 = p * PAIR_TOK + tch * 128
        nc.sync.dma_start(out_ap[row0 : row0 + 128, :], out_sb[:, :])


def build_kernel():
    nc = bacc.Bacc("TRN2", target_bir_lowering=False)
    x_t = nc.dram_tensor("x", [TOK_CORE, C], F32, kind="ExternalInput")
    wqkv_t = nc.dram_tensor("W_qkv", [C, 3 * C], F32, kind="ExternalInput")
    wproj_t = nc.dram_tensor("W_proj", [C, C], F32, kind="ExternalInput")
    bias_t = nc.dram_tensor("b_proj", [C], F32, kind="ExternalInput")
    out_t = nc.dram_tensor("out", [TOK_CORE, C], F32, kind="ExternalOutput")

    with TileContext(nc) as tc:
        with contextlib.ExitStack() as ctx:
            pools = {
                "const": ctx.enter_context(tc.tile_pool(name="const", bufs=1)),
                "stage": ctx.enter_context(tc.tile_pool(name="stage", bufs=2)),
                "xst": ctx.enter_context(tc.tile_pool(name="xst", bufs=2)),
                "xT": ctx.enter_context(tc.tile_pool(name="xT", bufs=2)),
                "x16": ctx.enter_context(tc.tile_pool(name="x16", bufs=1)),
                "qkT": ctx.enter_context(tc.tile_pool(name="qkT", bufs=2)),
                "va": ctx.enter_context(tc.tile_pool(name="va", bufs=2)),
                "es": ctx.enter_context(tc.tile_pool(name="es", bufs=2)),
                "esm": ctx.enter_context(tc.tile_pool(name="esm", bufs=2)),
                "attnT": ctx.enter_context(tc.tile_pool(name="attnT", bufs=2)),
                "out": ctx.enter_context(tc.tile_pool(name="out", bufs=3)),
                "poc": ctx.enter_context(tc.tile_pool(name="poc", bufs=5)),
                "bc": ctx.enter_context(tc.tile_pool(name="bc", bufs=5)),
                "tmp": ctx.enter_context(tc.tile_pool(name="tmp", bufs=6)),
                "rcpb": ctx.enter_context(tc.tile_pool(name="rcpb", bufs=4)),
                "big2": ctx.enter_context(
                    tc.tile_pool(name="big2", bufs=2, space="PSUM")
                ),
                "po": ctx.enter_context(
                    tc.tile_pool(name="po", bufs=4, space="PSUM")
                ),
            }
            const = pools["const"]

            wqkv16 = const.tile([128, KS, 3 * C], F16, tag="wqkv16")
            wproj16 = const.tile([128, KS, C], F16, tag="wproj16")
            bias_bc = const.tile([128, C], F32, tag="bias_bc")
            ident32 = const.tile([128, 128], F32, tag="ident32")
            make_identity(nc, ident32)
            ident16 = const.tile([128, 128], F16, tag="ident16")
            make_identity(nc, ident16)
            x0 = _load_x(nc, pools, x_t.ap(), 0)

            # stage W_qkv by 128-column blocks so the first qk matmul only
            # waits for one ~1.2MB block instead of the whole 7MB weight
            for fb in range(18):
                st = pools["stage"].tile([128, KS, 128], F32, tag="stage")
                nc.sync.dma_start(
                    st[:],
                    wqkv_t.ap()[:, fb * 128 : (fb + 1) * 128].rearrange(
                        "(k p) f -> p k f", p=128
                    ),
                )
                if fb % 2 == 0:
                    nc.scalar.copy(
                        wqkv16[:, :, fb * 128 : (fb + 1) * 128], st[:]
                    )
                else:
                    nc.vector.tensor_copy(
                        wqkv16[:, :, fb * 128 : (fb + 1) * 128], st[:]
                    )
            # software pipeline: transposes of pair p+1 are emitted between
            # batch 0 and batch 1 of pair p; wproj/bias staging is emitted
            # after the first batch so it does not crowd the startup
            xT_next = _emit_xT(nc, pools, ident16, x0)
            qkT_next = _emit_qk(nc, pools, wqkv16, xT_next)
            for p in range(NPAIR):
                xT = xT_next
                qkT = qkT_next
                attnT = pools["attnT"].tile([128, KS, PAIR_TOK], F16, tag="attnT")
                _emit_batch(nc, pools, wqkv16, xT, qkT, attnT, 0)
                if p == 0:
                    for ks in range(KS):
                        stp = pools["stage"].tile([128, C], F32, tag="stagep")
                        nc.sync.dma_start(
                            stp[:], wproj_t.ap()[ks * 128 : (ks + 1) * 128, :]
                        )
                        if ks % 2 == 0:
                            nc.scalar.copy(wproj16[:, ks, :], stp[:])
                        else:
                            nc.vector.tensor_copy(wproj16[:, ks, :], stp[:])
                    brow = pools["stage"].tile([1, C], F32, tag="stagep")
                    nc.sync.dma_start(brow[0:1, :], bias_t.ap().unsqueeze(0))
                    nc.gpsimd.partition_broadcast(bias_bc[:, :], brow[0:1, :])
                if p + 1 < NPAIR:
                    xn = _load_x(nc, pools, x_t.ap(), p + 1)
                    xT_next = _emit_xT(nc, pools, ident16, xn)
                _emit_batch(nc, pools, wqkv16, xT, qkT, attnT, N)
                if p + 1 < NPAIR:
                    qkT_next = _emit_qk(nc, pools, wqkv16, xT_next)
                _emit_proj(
                    nc, pools, wproj16, bias_bc, attnT, out_t.ap(), p,
                    (0, 1, 2, 3, 4),
                )

    nc.compile()
    return nc


@functools.cache
def _get_nc():
    return build_kernel()


def kernel(**inputs):
    x = np.ascontiguousarray(np.asarray(inputs["x"], dtype=np.float32))
    wqkv = np.ascontiguousarray(np.asarray(inputs["W_qkv"], dtype=np.float32))
    wproj = np.ascontiguousarray(np.asarray(inputs["W_proj"], dtype=np.float32))
    bias = np.ascontiguousarray(np.asarray(inputs["b_proj"], dtype=np.float32))
    t_h = int(inputs.get("t_h", 8))
    t_w = int(inputs.get("t_w", 8))
    assert t_h * t_w == 64, "kernel built for template length 64"
    assert x.shape == (B, N, C)

    nc = _get_nc()
    in_maps = [
        {
            "x": x[c * B_CORE : (c + 1) * B_CORE].reshape(TOK_CORE, C),
            "W_qkv": wqkv,
            "W_proj": wproj,
            "b_proj": bias,
        }
        for c in range(NCORES)
    ]
    res = run_bass_kernel_spmd(nc, in_maps, core_ids=list(range(NCORES)))
    out = np.concatenate(
        [r["out"].reshape(B_CORE, N, C) for r in res.results], axis=0
    )
    return out.astype(np.float32)


if __name__ == "__main__":
    _get_nc()
    print("kernel built OK")
